# revision 8
# baseline (speedup 1.0000x reference)
"""Trainium2 Bass kernel for nn_BezierParameterProcessor.

Data-parallel over batch: B=16 -> 2 batches per core on 8 cores.

The KDE gaussian over the [-1,1]^2 tensor-product grid is separable:
    exp(-((gx-px)^2+(gy-py)^2)/(2 bw^2)) = Ex[n,w] * Ey[n,h]
so density/field reduce to per-h-chunk matmuls contracting n:
    dens[h,w]    = sum_n Ey[n,h] *  Ex[n,w]
    field_c[h,w] = sum_n Ey[n,h] * (Ex[n,w] * c0*valid[n]*vecs[n,c])
This needs 2*N*256 exps per batch instead of N*65536, and the whole
[B,HW,N] intermediate never exists. sigmoid(z) = 0.5*tanh(z/2)+0.5 keeps
every ACT call in the single `exp_and_others` table set (exp, tanh,
square all live there -> one ACT_TABLE_LOAD).

fp16 is used for matmul operands (fp32 PSUM accumulation): the gaussian
factors live in [0,1] and the MLP activations are O(1), so fp16 costs
~5e-4 relative error while running the PE single-pass (fp32 matmuls
decompose into 2x LDWEIGHTS + 2x MATMUL at ~4x the cost).

Inputs are packed into two [128, F] blobs (one per dtype): 2 input DMAs.
Point coords are stored NEGATED so (g - p)^2 is one ACT Square with the
coord as per-partition bias.  All six [128,256] output maps of a batch
are built in one [128,1536] SBUF tile and shipped with a single DMA to a
partition-major scratch layout; the host untangles it while unsharding.
"""

import math

import numpy as np

import bass_rust
import concourse.bass as bass
import concourse.tile as tile
from concourse import mybir
from concourse.bass_utils import run_bass_kernel_spmd

H = W = 256
HID = 128
B = 16
N = 128  # points per batch (C*P = 16*8)
NCORES = 8
BS = B // NCORES  # batches per core = 2

FP32 = mybir.dt.float32
FP16 = mybir.dt.float16

# blob32 column layout: gx | gy | npc (negated coords) | bf | b3
C_GX, C_GY, C_PC, C_BF, C_B3 = 0, 256, 512, 516, 518
NC32 = 519
# blob16 column layout: w1t | w2t | w3t | wft | x0
C_W1, C_W2, C_W3, C_WF, C_X0 = 0, 64, 192, 320, 322
NC16 = 578

LAST_RESULT = None  # BassKernelResults of the most recent run (for profiling)


def _legalize_sync_waits(nc):
    """Split inline sem-waits beyond one per instruction onto preceding
    same-engine NOPs.  This toolchain snapshot's Tile wait-assignment can
    put 2+ waits on opcodes whose walrus codegen struct has a single
    sync-wait slot (Drain, branches, Matmult LDW), which fails compile
    with "Too many sync wait commands"."""
    for f in nc.m.functions:
        for blk in f.blocks:
            insts = blk.instructions  # live list
            idx = 0
            while idx < len(insts):
                inst = insts[idx]
                si = inst.sync_info
                if si is None or len(si.on_wait) <= 1:
                    idx += 1
                    continue
                waits = list(si.on_wait)
                extra, keep = waits[:-1], waits[-1:]
                nops = []
                for wt in extra:
                    before = {b.name: len(b.instructions) for b in f.blocks}
                    n = nc.engines[inst.engine].nop(hint="wait_split", nofuse=True)
                    for b in f.blocks:
                        if len(b.instructions) != before.get(b.name, 0):
                            assert b.instructions[-1].name == n.ins.name
                            b.instructions.pop()
                            break
                    n.ins.sync_info = bass_rust.SyncInfo(on_wait=[wt], on_update=[])
                    nops.append(n.ins)
                si.on_wait = keep
                inst.sync_info = si
                insts[idx:idx] = nops
                idx += len(nops) + 1


def _build(neg_inv2bw2, c0, sig_half_scale, sig_half_bias):
    AL = mybir.AluOpType
    ACT = mybir.ActivationFunctionType
    nc = bass.Bass("TRN2", target_bir_lowering=False)

    b32_d = nc.declare_dram_parameter("b32", [128, NC32], FP32, isOutput=False)
    b16_d = nc.declare_dram_parameter("b16", [128, NC16], FP16, isOutput=False)
    # scratch layout: [b][partition][1536] = dens(2x256) | f0(2x256) | f1(2x256)
    out_d = nc.declare_dram_parameter("out_o", [BS, 128, 1536], FP32, isOutput=True)

    with tile.TileContext(nc) as tc:
        with (
            tc.tile_pool(name="const", bufs=1) as cpool,
            tc.tile_pool(name="work", bufs=2) as wpool,
            tc.tile_pool(name="mlpps", bufs=2, space="PSUM") as mpsum,
            tc.tile_pool(name="redps", bufs=2, space="PSUM") as rpsum,
        ):
            b32 = cpool.tile([128, NC32], FP32, tag="b32")
            nc.sync.dma_start(b32[:], b32_d[:])
            b16 = cpool.tile([128, NC16], FP16, tag="b16")
            nc.gpsimd.dma_start(b16[:], b16_d[:])

            gx = b32[:, C_GX : C_GX + W]
            gy = b32[:, C_GY : C_GY + H]
            bf = b32[:, C_BF : C_BF + 2]
            b3c = b32[:, C_B3 : C_B3 + 1]
            w1 = b16[:3, C_W1 : C_W1 + 64]
            w2 = b16[:65, C_W2 : C_W2 + HID]
            w3 = b16[:, C_W3 : C_W3 + HID]
            wf = b16[:, C_WF : C_WF + 2]
            x0 = b16[:3, C_X0 : C_X0 + BS * N]

            sigb = cpool.tile([128, 1], FP32, tag="sigb")
            nc.gpsimd.memset(sigb[:], sig_half_bias)

            # ---- point-encoder MLP, both batches' points at once ----
            # features on partitions, points on the free dim
            ps1 = mpsum.tile([64, BS * N], FP32, tag="mlp")
            nc.tensor.matmul(ps1[:], w1, x0)  # K=3 (x, y, 1)
            h1 = wpool.tile([65, BS * N], FP16, tag="h1")
            nc.vector.tensor_scalar(h1[:64, :], ps1[:], 0.0, None, AL.max)  # relu
            nc.gpsimd.memset(h1[64:65, :], 1.0)  # bias row for layer 2

            ps2 = mpsum.tile([HID, BS * N], FP32, tag="mlp")
            nc.tensor.matmul(ps2[:], w2, h1[:])  # K=65
            h2 = wpool.tile([HID, BS * N], FP16, tag="h2")
            nc.vector.tensor_scalar(h2[:], ps2[:], 0.0, None, AL.max)  # relu

            ps3 = mpsum.tile([HID, BS * N], FP32, tag="mlp")
            nc.tensor.matmul(ps3[:], w3, h2[:])
            enc = wpool.tile([HID, BS * N], FP16, tag="enc")
            nc.vector.tensor_scalar(enc[:], ps3[:], b3c, None, AL.add)  # + b3

            # ---- per-batch: u[n,c] = c0*valid[n]*vecs[n,c] ----
            ubs = []
            for b in range(BS):
                psv = mpsum.tile([N, 2], FP32, tag="mlp")
                nc.tensor.matmul(psv[:], enc[:, b * N : (b + 1) * N], wf)
                vb = wpool.tile([N, 2], FP32, tag="vb")
                nc.vector.tensor_tensor(vb[:], psv[:], bf, AL.add)

                npx = b32[:, C_PC + 2 * b : C_PC + 2 * b + 1]
                npy = b32[:, C_PC + 2 * b + 1 : C_PC + 2 * b + 2]
                vc = wpool.tile([N, 2], FP32, tag="vc")
                # |px| = max(-px, px) in one two-op tensor_scalar
                nc.vector.tensor_scalar(vc[:, 0:1], npx, -1.0, npx, AL.mult, AL.max)
                nc.vector.tensor_scalar(vc[:, 1:2], npy, -1.0, npy, AL.mult, AL.max)
                nc.vector.tensor_tensor(vc[:, 0:1], vc[:, 0:1], vc[:, 1:2], AL.max)
                # (max(|px|,|py|) > 1e-8) * c0
                nc.vector.tensor_scalar(
                    vc[:, 0:1], vc[:, 0:1], 1e-8, c0, AL.is_gt, AL.mult
                )
                ub = wpool.tile([N, 2], FP32, tag="ub")
                nc.vector.tensor_scalar(ub[:], vb[:], vc[:, 0:1], None, AL.mult)
                ubs.append(ub)

            # ---- separable gaussian factors ----
            # coords stored negated: (gx - px)^2 = Square(gx + npx)
            txs = wpool.tile([128, BS * W], FP32, tag="txs")
            tys = wpool.tile([128, BS * H], FP32, tag="tys")
            for b in range(BS):
                npx = b32[:, C_PC + 2 * b : C_PC + 2 * b + 1]
                npy = b32[:, C_PC + 2 * b + 1 : C_PC + 2 * b + 2]
                nc.scalar.activation(
                    txs[:, b * W : (b + 1) * W], gx, ACT.Square, bias=npx
                )
                nc.scalar.activation(
                    tys[:, b * H : (b + 1) * H], gy, ACT.Square, bias=npy
                )

            ey = wpool.tile([128, BS * H], FP16, tag="ey")
            nc.scalar.activation(ey[:], tys[:], ACT.Exp, scale=neg_inv2bw2)
            # exa_b = [ Ex_b | Ex_b * u0_b ]   exu1_b = Ex_b * u1_b
            exas, exu1s = [], []
            for b in range(BS):
                exa = wpool.tile([128, 2 * W], FP16, tag="exa")
                nc.scalar.activation(
                    exa[:, 0:W],
                    txs[:, b * W : (b + 1) * W],
                    ACT.Exp,
                    scale=neg_inv2bw2,
                )
                nc.vector.tensor_scalar(
                    exa[:, W : 2 * W], exa[:, 0:W], ubs[b][:, 0:1], None, AL.mult
                )
                exu1 = wpool.tile([128, W], FP16, tag="exu1")
                nc.vector.tensor_scalar(
                    exu1[:], exa[:, 0:W], ubs[b][:, 1:2], None, AL.mult
                )
                exas.append(exa)
                exu1s.append(exu1)

            # ---- reductions + epilogue + one store per batch ----
            for b in range(BS):
                ps = rpsum.tile([128, 1536], FP32, tag="ps")  # 3 banks
                for ch in range(2):
                    lhs = ey[:, b * H + ch * 128 : b * H + (ch + 1) * 128]
                    o = ch * W
                    nc.tensor.matmul(ps[:, o : o + W], lhs, exas[b][:, 0:W])
                    nc.tensor.matmul(
                        ps[:, 512 + o : 512 + o + W], lhs, exas[b][:, W : 2 * W]
                    )
                    nc.tensor.matmul(ps[:, 1024 + o : 1024 + o + W], lhs, exu1s[b][:])

                obuf = wpool.tile([128, 1536], FP32, tag="obuf")
                # sigmoid(s*x - thr) = 0.5*tanh(s/2*x - thr/2) + 0.5
                nc.scalar.activation(
                    obuf[:, 0:512],
                    ps[:, 0:512],
                    ACT.Tanh,
                    scale=sig_half_scale,
                    bias=sigb[:],
                )
                nc.scalar.activation(obuf[:, 512:1536], ps[:, 512:1536], ACT.Tanh)
                nc.vector.tensor_scalar(
                    obuf[:, 0:512], obuf[:, 0:512], 0.5, 0.5, AL.mult, AL.add
                )
                (nc.sync if b == 0 else nc.gpsimd).dma_start(out_d[b], obuf[:])

    _legalize_sync_waits(nc)
    return nc


def kernel(
    bezier_points,
    W1,
    b1,
    W2,
    b2,
    W3,
    b3,
    Wf,
    bf,
    kde_bandwidth,
    density_threshold,
    trace=False,
):
    global LAST_RESULT
    f32, f16 = np.float32, np.float16
    pts = np.asarray(bezier_points, f32).reshape(B, N, 2)
    W1, b1 = np.asarray(W1, f32), np.asarray(b1, f32)
    W2, b2 = np.asarray(W2, f32), np.asarray(b2, f32)
    W3, b3 = np.asarray(W3, f32), np.asarray(b3, f32)
    Wf, bf = np.asarray(Wf, f32), np.asarray(bf, f32)

    bw = max(float(np.float32(kde_bandwidth)), 1e-5)
    thr = float(np.float32(density_threshold))
    neg_inv2bw2 = -1.0 / (2.0 * bw * bw)
    c0 = math.exp(-1e-8 / (2.0 * bw * bw))
    s = 1.0 / (N * bw * math.sqrt(2.0 * math.pi))
    nc = _build(neg_inv2bw2, c0, 0.5 * s, -0.5 * thr)

    # host-side input marshalling (grid constants + weight transposes)
    blob32 = np.zeros((128, NC32), f32)
    blob32[:, C_GX : C_GX + W] = np.linspace(-1.0, 1.0, W, dtype=f32)
    blob32[:, C_GY : C_GY + H] = np.linspace(-1.0, 1.0, H, dtype=f32)
    blob32[:, C_BF : C_BF + 2] = bf
    blob32[:, C_B3] = b3
    blob16 = np.zeros((128, NC16), f16)
    blob16[:3, C_W1 : C_W1 + 64] = np.vstack([W1.T, b1[None, :]])
    blob16[:65, C_W2 : C_W2 + HID] = np.vstack([W2.T, b2[None, :]])
    blob16[:, C_W3 : C_W3 + HID] = W3.T
    blob16[:, C_WF : C_WF + 2] = Wf.T

    in_maps = []
    for i in range(NCORES):
        sh = pts[i * BS : (i + 1) * BS]  # [BS, N, 2]
        c32 = blob32.copy()
        for b in range(BS):
            c32[:, C_PC + 2 * b] = -sh[b, :, 0]
            c32[:, C_PC + 2 * b + 1] = -sh[b, :, 1]
        c16 = blob16.copy()
        c16[0, C_X0 : C_X0 + BS * N] = sh[..., 0].reshape(-1)
        c16[1, C_X0 : C_X0 + BS * N] = sh[..., 1].reshape(-1)
        c16[2, C_X0 : C_X0 + BS * N] = 1.0
        in_maps.append({"b32": c32, "b16": c16})

    res = run_bass_kernel_spmd(nc, in_maps, list(range(NCORES)), trace=trace)
    LAST_RESULT = res

    density = np.empty((B, 1, H, W), f32)
    field = np.empty((B, 2, H, W), f32)
    for i in range(NCORES):
        scr = res.results[i]["out_o"]  # [BS, 128, 1536]
        maps = scr.reshape(BS, 128, 3, 2, W).transpose(2, 0, 3, 1, 4)
        # maps[m, b, ch, p, w] with h = ch*128 + p
        density[i * BS : (i + 1) * BS, 0] = maps[0].reshape(BS, H, W)
        field[i * BS : (i + 1) * BS, 0] = maps[1].reshape(BS, H, W)
        field[i * BS : (i + 1) * BS, 1] = maps[2].reshape(BS, H, W)
    return density, field


# revision 10
# speedup vs baseline: 1.1325x; 1.1325x over previous
"""Trainium2 Bass kernel for nn_BezierParameterProcessor.

Data-parallel over batch: B=16 -> 2 batches per core on 8 cores.

The KDE gaussian over the [-1,1]^2 tensor-product grid is separable:
    exp(-((gx-px)^2+(gy-py)^2)/(2 bw^2)) = Ex[n,w] * Ey[n,h]
so density/field reduce to per-h-chunk matmuls contracting n:
    dens[h,w]    = sum_n Ey[n,h] *  Ex[n,w]
    field_c[h,w] = sum_n Ey[n,h] * (Ex[n,w] * c0*valid[n]*vecs[n,c])
This needs 2*N*256 exps per batch instead of N*65536, and the whole
[B,HW,N] intermediate never exists. sigmoid(z) = 0.5*tanh(z/2)+0.5 keeps
every ACT call in the single `exp_and_others` table set (exp, tanh,
square all live there -> one ACT_TABLE_LOAD).

fp16 is used for matmul operands (fp32 PSUM accumulation): the gaussian
factors live in [0,1] and the MLP activations are O(1), so fp16 costs
~5e-4 relative error while running the PE single-pass (fp32 matmuls
decompose into 2x LDWEIGHTS + 2x MATMUL at ~4x the cost).

Inputs are packed into two [128, F] blobs (one per dtype): 2 input DMAs.
Point coords are stored NEGATED so (g - p)^2 is one ACT Square with the
coord as per-partition bias.  All six [128,256] output maps of a batch
are built in one [128,1536] SBUF tile and shipped with a single DMA to a
partition-major scratch layout; the host untangles it while unsharding.
"""

import math

import numpy as np

import bass_rust
import concourse.bass as bass
import concourse.tile as tile
from concourse import mybir
from concourse.bass_utils import run_bass_kernel_spmd

H = W = 256
HID = 128
B = 16
N = 128  # points per batch (C*P = 16*8)
NCORES = 8
BS = B // NCORES  # batches per core = 2

FP32 = mybir.dt.float32
FP16 = mybir.dt.float16

# blob32 column layout: gx | gy | npc (negated coords) | bf | b3
C_GX, C_GY, C_PC, C_BF, C_B3 = 0, 256, 512, 516, 518
NC32 = 519
# blob16 column layout: w1t | w2t | w3t | wft | x0
C_W1, C_W2, C_W3, C_WF, C_X0 = 0, 64, 192, 320, 322
NC16 = 578

LAST_RESULT = None  # BassKernelResults of the most recent run (for profiling)


def _legalize_sync_waits(nc):
    """Split inline sem-waits beyond one per instruction onto preceding
    same-engine NOPs.  This toolchain snapshot's Tile wait-assignment can
    put 2+ waits on opcodes whose walrus codegen struct has a single
    sync-wait slot (Drain, branches, Matmult LDW), which fails compile
    with "Too many sync wait commands"."""
    for f in nc.m.functions:
        for blk in f.blocks:
            insts = blk.instructions  # live list
            idx = 0
            while idx < len(insts):
                inst = insts[idx]
                si = inst.sync_info
                if si is None or len(si.on_wait) <= 1:
                    idx += 1
                    continue
                waits = list(si.on_wait)
                extra, keep = waits[:-1], waits[-1:]
                nops = []
                for wt in extra:
                    before = {b.name: len(b.instructions) for b in f.blocks}
                    n = nc.engines[inst.engine].nop(hint="wait_split", nofuse=True)
                    for b in f.blocks:
                        if len(b.instructions) != before.get(b.name, 0):
                            assert b.instructions[-1].name == n.ins.name
                            b.instructions.pop()
                            break
                    n.ins.sync_info = bass_rust.SyncInfo(on_wait=[wt], on_update=[])
                    nops.append(n.ins)
                si.on_wait = keep
                inst.sync_info = si
                insts[idx:idx] = nops
                idx += len(nops) + 1


def _build(neg_inv2bw2, c0, sig_half_scale, sig_half_bias):
    AL = mybir.AluOpType
    ACT = mybir.ActivationFunctionType
    nc = bass.Bass("TRN2", target_bir_lowering=False)

    b32_d = nc.declare_dram_parameter("b32", [128, NC32], FP32, isOutput=False)
    b16_d = nc.declare_dram_parameter("b16", [128, NC16], FP16, isOutput=False)
    # scratch layout: [b][partition][1536] = dens(2x256) | f0(2x256) | f1(2x256)
    out_d = nc.declare_dram_parameter("out_o", [BS, 128, 1536], FP32, isOutput=True)

    with tile.TileContext(nc) as tc:
        with (
            tc.tile_pool(name="const", bufs=1) as cpool,
            tc.tile_pool(name="work", bufs=2) as wpool,
            tc.tile_pool(name="mlpps", bufs=2, space="PSUM") as mpsum,
            tc.tile_pool(name="redps", bufs=2, space="PSUM") as rpsum,
        ):
            b32 = cpool.tile([128, NC32], FP32, tag="b32")
            nc.scalar.dma_start(b32[:], b32_d[:])
            b16 = cpool.tile([128, NC16], FP16, tag="b16")
            nc.sync.dma_start(b16[:], b16_d[:])

            gx = b32[:, C_GX : C_GX + W]
            gy = b32[:, C_GY : C_GY + H]
            bf = b32[:, C_BF : C_BF + 2]
            b3c = b32[:, C_B3 : C_B3 + 1]
            w1 = b16[:3, C_W1 : C_W1 + 64]
            w2 = b16[:65, C_W2 : C_W2 + HID]
            w3 = b16[:, C_W3 : C_W3 + HID]
            wf = b16[:, C_WF : C_WF + 2]
            x0 = b16[:3, C_X0 : C_X0 + BS * N]

            sigb = cpool.tile([128, 1], FP32, tag="sigb")
            nc.gpsimd.memset(sigb[:], sig_half_bias)

            # ---- point-encoder MLP, both batches' points at once ----
            # features on partitions, points on the free dim
            ps1 = mpsum.tile([64, BS * N], FP32, tag="mlp")
            nc.tensor.matmul(ps1[:], w1, x0)  # K=3 (x, y, 1)
            h1 = wpool.tile([65, BS * N], FP16, tag="h1")
            nc.vector.tensor_scalar(h1[:64, :], ps1[:], 0.0, None, AL.max)  # relu
            nc.gpsimd.memset(h1[64:65, :], 1.0)  # bias row for layer 2

            ps2 = mpsum.tile([HID, BS * N], FP32, tag="mlp")
            nc.tensor.matmul(ps2[:], w2, h1[:])  # K=65
            h2 = wpool.tile([HID, BS * N], FP16, tag="h2")
            nc.vector.tensor_scalar(h2[:], ps2[:], 0.0, None, AL.max)  # relu

            ps3 = mpsum.tile([HID, BS * N], FP32, tag="mlp")
            nc.tensor.matmul(ps3[:], w3, h2[:])
            enc = wpool.tile([HID, BS * N], FP16, tag="enc")
            nc.vector.tensor_scalar(enc[:], ps3[:], b3c, None, AL.add)  # + b3

            # ---- per-batch: u[n,c] = c0*valid[n]*vecs[n,c] ----
            ubs = []
            for b in range(BS):
                psv = mpsum.tile([N, 2], FP32, tag="mlp")
                nc.tensor.matmul(psv[:], enc[:, b * N : (b + 1) * N], wf)
                vb = wpool.tile([N, 2], FP32, tag="vb")
                nc.vector.tensor_tensor(vb[:], psv[:], bf, AL.add)

                npx = b32[:, C_PC + 2 * b : C_PC + 2 * b + 1]
                npy = b32[:, C_PC + 2 * b + 1 : C_PC + 2 * b + 2]
                vc = wpool.tile([N, 2], FP32, tag="vc")
                # |px| = max(-px, px) in one two-op tensor_scalar
                nc.vector.tensor_scalar(vc[:, 0:1], npx, -1.0, npx, AL.mult, AL.max)
                nc.vector.tensor_scalar(vc[:, 1:2], npy, -1.0, npy, AL.mult, AL.max)
                nc.vector.tensor_tensor(vc[:, 0:1], vc[:, 0:1], vc[:, 1:2], AL.max)
                # (max(|px|,|py|) > 1e-8) * c0
                nc.vector.tensor_scalar(
                    vc[:, 0:1], vc[:, 0:1], 1e-8, c0, AL.is_gt, AL.mult
                )
                ub = wpool.tile([N, 2], FP32, tag="ub")
                nc.vector.tensor_scalar(ub[:], vb[:], vc[:, 0:1], None, AL.mult)
                ubs.append(ub)

            # ---- separable gaussian factors ----
            # coords stored negated: (gx - px)^2 = Square(gx + npx)
            txs = wpool.tile([128, BS * W], FP32, tag="txs")
            tys = wpool.tile([128, BS * H], FP32, tag="tys")
            for b in range(BS):
                npx = b32[:, C_PC + 2 * b : C_PC + 2 * b + 1]
                npy = b32[:, C_PC + 2 * b + 1 : C_PC + 2 * b + 2]
                nc.scalar.activation(
                    txs[:, b * W : (b + 1) * W], gx, ACT.Square, bias=npx
                )
                nc.scalar.activation(
                    tys[:, b * H : (b + 1) * H], gy, ACT.Square, bias=npy
                )

            ey = wpool.tile([128, BS * H], FP16, tag="ey")
            nc.scalar.activation(ey[:], tys[:], ACT.Exp, scale=neg_inv2bw2)
            # exa_b = [ Ex_b | Ex_b * u0_b ]   exu1_b = Ex_b * u1_b
            exas, exu1s = [], []
            for b in range(BS):
                exa = wpool.tile([128, 2 * W], FP16, tag="exa")
                nc.scalar.activation(
                    exa[:, 0:W],
                    txs[:, b * W : (b + 1) * W],
                    ACT.Exp,
                    scale=neg_inv2bw2,
                )
                nc.vector.tensor_scalar(
                    exa[:, W : 2 * W], exa[:, 0:W], ubs[b][:, 0:1], None, AL.mult
                )
                exu1 = wpool.tile([128, W], FP16, tag="exu1")
                nc.vector.tensor_scalar(
                    exu1[:], exa[:, 0:W], ubs[b][:, 1:2], None, AL.mult
                )
                exas.append(exa)
                exu1s.append(exu1)

            # ---- reductions + epilogue + one store per batch ----
            for b in range(BS):
                ps = rpsum.tile([128, 1536], FP32, tag="ps")  # 3 banks
                for ch in range(2):
                    lhs = ey[:, b * H + ch * 128 : b * H + (ch + 1) * 128]
                    o = ch * W
                    nc.tensor.matmul(ps[:, o : o + W], lhs, exas[b][:, 0:W])
                    nc.tensor.matmul(
                        ps[:, 512 + o : 512 + o + W], lhs, exas[b][:, W : 2 * W]
                    )
                    nc.tensor.matmul(ps[:, 1024 + o : 1024 + o + W], lhs, exu1s[b][:])

                obuf = wpool.tile([128, 1536], FP32, tag="obuf")
                # sigmoid(s*x - thr) = 0.5*tanh(s/2*x - thr/2) + 0.5
                nc.scalar.activation(
                    obuf[:, 0:512],
                    ps[:, 0:512],
                    ACT.Tanh,
                    scale=sig_half_scale,
                    bias=sigb[:],
                )
                nc.scalar.activation(obuf[:, 512:1536], ps[:, 512:1536], ACT.Tanh)
                nc.vector.tensor_scalar(
                    obuf[:, 0:512], obuf[:, 0:512], 0.5, 0.5, AL.mult, AL.add
                )
                nc.sync.dma_start(out_d[b], obuf[:])

    _legalize_sync_waits(nc)
    return nc


def kernel(
    bezier_points,
    W1,
    b1,
    W2,
    b2,
    W3,
    b3,
    Wf,
    bf,
    kde_bandwidth,
    density_threshold,
    trace=False,
):
    global LAST_RESULT
    f32, f16 = np.float32, np.float16
    pts = np.asarray(bezier_points, f32).reshape(B, N, 2)
    W1, b1 = np.asarray(W1, f32), np.asarray(b1, f32)
    W2, b2 = np.asarray(W2, f32), np.asarray(b2, f32)
    W3, b3 = np.asarray(W3, f32), np.asarray(b3, f32)
    Wf, bf = np.asarray(Wf, f32), np.asarray(bf, f32)

    bw = max(float(np.float32(kde_bandwidth)), 1e-5)
    thr = float(np.float32(density_threshold))
    neg_inv2bw2 = -1.0 / (2.0 * bw * bw)
    c0 = math.exp(-1e-8 / (2.0 * bw * bw))
    s = 1.0 / (N * bw * math.sqrt(2.0 * math.pi))
    nc = _build(neg_inv2bw2, c0, 0.5 * s, -0.5 * thr)

    # host-side input marshalling (grid constants + weight transposes)
    blob32 = np.zeros((128, NC32), f32)
    blob32[:, C_GX : C_GX + W] = np.linspace(-1.0, 1.0, W, dtype=f32)
    blob32[:, C_GY : C_GY + H] = np.linspace(-1.0, 1.0, H, dtype=f32)
    blob32[:, C_BF : C_BF + 2] = bf
    blob32[:, C_B3] = b3
    blob16 = np.zeros((128, NC16), f16)
    blob16[:3, C_W1 : C_W1 + 64] = np.vstack([W1.T, b1[None, :]])
    blob16[:65, C_W2 : C_W2 + HID] = np.vstack([W2.T, b2[None, :]])
    blob16[:, C_W3 : C_W3 + HID] = W3.T
    blob16[:, C_WF : C_WF + 2] = Wf.T

    in_maps = []
    for i in range(NCORES):
        sh = pts[i * BS : (i + 1) * BS]  # [BS, N, 2]
        c32 = blob32.copy()
        for b in range(BS):
            c32[:, C_PC + 2 * b] = -sh[b, :, 0]
            c32[:, C_PC + 2 * b + 1] = -sh[b, :, 1]
        c16 = blob16.copy()
        c16[0, C_X0 : C_X0 + BS * N] = sh[..., 0].reshape(-1)
        c16[1, C_X0 : C_X0 + BS * N] = sh[..., 1].reshape(-1)
        c16[2, C_X0 : C_X0 + BS * N] = 1.0
        in_maps.append({"b32": c32, "b16": c16})

    res = run_bass_kernel_spmd(nc, in_maps, list(range(NCORES)), trace=trace)
    LAST_RESULT = res

    density = np.empty((B, 1, H, W), f32)
    field = np.empty((B, 2, H, W), f32)
    for i in range(NCORES):
        scr = res.results[i]["out_o"]  # [BS, 128, 1536]
        maps = scr.reshape(BS, 128, 3, 2, W).transpose(2, 0, 3, 1, 4)
        # maps[m, b, ch, p, w] with h = ch*128 + p
        density[i * BS : (i + 1) * BS, 0] = maps[0].reshape(BS, H, W)
        field[i * BS : (i + 1) * BS, 0] = maps[1].reshape(BS, H, W)
        field[i * BS : (i + 1) * BS, 1] = maps[2].reshape(BS, H, W)
    return density, field
